# revision 31
# baseline (speedup 1.0000x reference)
"""Trainium2 Bass kernel for nn_AggregPolicy (GNN message passing / GRU chain).

Strategy:
  - Pure data parallelism: 524288 rows split across 8 cores (65536 each).
  - Feature-major on-chip layout: state s = [hj0..hj6, hm] (32 features) on
    partitions, batch on the free dim. 4 batch subgroups stacked on partitions
    (partition 32q+s) so elementwise ops use all 128 lanes.
  - Each GRU message-passing step's full linear algebra is a 32->128 linear map
    (neighbor structure folded into a banded weight matrix). Executed as 16
    small matmuls (K=32, M=32) with tile_position packing, writing gate-type-
    major PSUM banks: R | Z | INN | HN, each [128, 512].
  - Nonlinearities: ACT sigmoid/tanh with fused per-partition bias; DVE/GPSIMD
    for the remaining pointwise ops.
  - Iteration 1 consumes x directly (initial Linear layers folded into the
    first step's weights); final Linear folded into 4 output matmuls; final
    bias + layout restore on host.
"""

import sys
import numpy as np

for _p in ("/opt/trn_rl_repo",):
    if _p not in sys.path:
        sys.path.append(_p)

import ml_dtypes
from contextlib import ExitStack

import concourse.bass as bass
import concourse.bacc as bacc
import concourse.tile as tile
from concourse.tile import add_dep_helper
from concourse import mybir
from concourse.bass_utils import run_bass_kernel_spmd

BF16 = ml_dtypes.bfloat16
AF = mybir.ActivationFunctionType
ALU = mybir.AluOpType

N_CORES = 8
B = 524288
BC = B // N_CORES          # rows per core = 65536
NSUB = 4                   # batch subgroups stacked on partitions
NCOL = BC // NSUB          # free-dim columns per subgroup = 16384
CT = 512                   # columns per supertile (one PSUM bank)
NT = NCOL // CT            # 32 supertiles
H = 4
NU = 8                     # 7 joints + master
S = 32                     # state features


def _gate_blocks(p):
    """Build the 32->128 banded linear map for one message-passing step.

    Returns W (gate-major blocks) [4][32, 32] mapping state->gates and the
    four per-partition bias vectors (within one 32-wide subgroup block).
    Gate blocks: 0=R(sum), 1=Z(sum), 2=INN (input side of n), 3=HN (hidden
    side of n, bias excluded -- applied via STT scalar).
    State layout: [hj0(4) .. hj6(4), hm(4)].
    """
    Wih_j, Whh_j = p["Wih_j"], p["Whh_j"]
    Wih_m, Whh_m = p["Wih_m"], p["Whh_m"]
    W = [np.zeros((S, S), np.float64) for _ in range(4)]

    def st(u):  # state slice of unit u
        return slice(4 * u, 4 * u + 4)

    for u in range(7):
        left = None if u == 0 else st(u - 1)   # u==0 -> hm
        right = None if u == 6 else st(u + 1)  # u==6 -> zero
        for g, rows in ((0, slice(0, 4)), (1, slice(4, 8))):
            # sum gates: Wih(left,right) + Whh(self)
            Wl = Wih_j[rows, 0:4]
            Wr = Wih_j[rows, 4:8]
            Wh = Whh_j[rows, :]
            tgt = W[g][st(u), :]
            if left is None:
                tgt[:, 28:32] += Wl
            else:
                tgt[:, left] += Wl
            if right is not None:
                tgt[:, right] += Wr
            tgt[:, st(u)] += Wh
        # INN: input side only
        rows = slice(8, 12)
        tgt = W[2][st(u), :]
        if u == 0:
            tgt[:, 28:32] += Wih_j[rows, 0:4]
        else:
            tgt[:, st(u - 1)] += Wih_j[rows, 0:4]
        if u != 6:
            tgt[:, st(u + 1)] += Wih_j[rows, 4:8]
        # HN: hidden side only
        W[3][st(u), st(u)] += Whh_j[rows, :]

    # master unit (index 7, state rows 28:32); input = hj0, hidden = hm
    for g, rows in ((0, slice(0, 4)), (1, slice(4, 8))):
        W[g][28:32, 0:4] += Wih_m[rows, :]
        W[g][28:32, 28:32] += Whh_m[rows, :]
    W[2][28:32, 0:4] += Wih_m[8:12, :]
    W[3][28:32, 28:32] += Whh_m[8:12, :]

    def unit_bias(vec_j, vec_m, rows):
        b = np.zeros(S, np.float64)
        for u in range(7):
            b[st(u)] = vec_j[rows]
        b[28:32] = vec_m[rows]
        return b

    br = unit_bias(p["bih_j"], p["bih_m"], slice(0, 4)) + unit_bias(
        p["bhh_j"], p["bhh_m"], slice(0, 4))
    bz = unit_bias(p["bih_j"], p["bih_m"], slice(4, 8)) + unit_bias(
        p["bhh_j"], p["bhh_m"], slice(4, 8))
    binn = unit_bias(p["bih_j"], p["bih_m"], slice(8, 12))
    bhn = unit_bias(p["bhh_j"], p["bhh_m"], slice(8, 12))
    return W, (br, bz, binn, bhn)


def _a0_ext(p):
    """[32, 19] initial-linear map: state0 = A0e @ [x(18); 1]."""
    A = np.zeros((S, 19), np.float64)
    Wj, bj, Wm, bm = p["Wj"], p["bj"], p["Wm"], p["bm"]
    for u in range(7):
        A[4 * u:4 * u + 4, 4 + u] = Wj[:, 0]
        A[4 * u:4 * u + 4, 11 + u] = Wj[:, 1]
        A[4 * u:4 * u + 4, 18] = bj
    A[28:32, 0:4] = Wm
    A[28:32, 18] = bm
    return A


def _host_weights(inputs):
    p = {k: np.asarray(v, np.float64) for k, v in inputs.items() if k != "x"}
    W, (br, bz, binn, bhn) = _gate_blocks(p)
    A0e = _a0_ext(p)

    # wtb [128,128]: rows 32q+k (k<32) = state idx, cols 32g+m = gate out m of block g
    wtb = np.zeros((128, 128), np.float64)
    # wt1 [128,128]: iteration-1 gate weights consuming xe(19) directly
    wt1 = np.zeros((128, 128), np.float64)
    # a0t: diag blocks for S0 psum (iter-1 blend h operand)
    a0t = np.zeros((128, 128), np.float64)
    # wat: diag blocks for output linear (state -> 7 activations)
    wat = np.zeros((128, 128), np.float64)
    Wa = p["Wa"]  # [1, 4]
    for q in range(4):
        r0 = 32 * q
        for g in range(4):
            wtb[r0:r0 + 32, 32 * g:32 * g + 32] = W[g].T
            W1g = W[g] @ A0e  # [32, 19]
            wt1[r0:r0 + 19, 32 * g:32 * g + 32] = W1g.T
        a0t[r0:r0 + 19, r0:r0 + 32] = A0e.T
        for u in range(7):
            wat[r0 + 4 * u:r0 + 4 * u + 4, r0 + u] = Wa[0, :]

    def bias128(v):
        return np.tile(v, 4).astype(np.float32).reshape(128, 1)

    return {
        "wtb": wtb.astype(BF16), "wt1": wt1.astype(BF16),
        "a0t": a0t.astype(BF16), "wat": wat.astype(BF16),
        "br": bias128(br), "bz": bias128(bz),
        "binn": bias128(binn), "bhn": bias128(bhn),
    }, float(np.asarray(inputs["ba"]).reshape(-1)[0])


def _host_x(x):
    """x [B,18] fp32 -> per-core [128, NCOL] bf16 (partition 32q+k, k<19)."""
    xs = []
    for c in range(N_CORES):
        xc = np.asarray(x[c * BC:(c + 1) * BC], np.float32)
        arr = np.zeros((4, 32, NCOL), np.float32)
        arr[:, 0:18, :] = xc.reshape(4, NCOL, 18).transpose(0, 2, 1)
        arr[:, 18, :] = 1.0
        xs.append(arr.reshape(128, NCOL).astype(BF16))
    return xs


def _build_program(ncol=NCOL, nt=NT, n_iters=7):
    nc = bacc.Bacc("TRN2", target_bir_lowering=False, debug=False,
                   num_devices=N_CORES)
    f32 = mybir.dt.float32
    bf16 = mybir.dt.bfloat16

    xd = nc.dram_tensor("x_il", [128, ncol], bf16, kind="ExternalInput").ap()
    wtbd = nc.dram_tensor("wtb", [128, 128], bf16, kind="ExternalInput").ap()
    wt1d = nc.dram_tensor("wt1", [128, 128], bf16, kind="ExternalInput").ap()
    a0td = nc.dram_tensor("a0t", [128, 128], bf16, kind="ExternalInput").ap()
    watd = nc.dram_tensor("wat", [128, 128], bf16, kind="ExternalInput").ap()
    biasd = {k: nc.dram_tensor(k, [128, 1], f32, kind="ExternalInput").ap()
             for k in ("br", "bz", "binn", "bhn")}
    yd = nc.dram_tensor("y", [28, ncol], f32, kind="ExternalOutput").ap()

    with tile.TileContext(nc) as tc, ExitStack() as ctx:
        cpool = ctx.enter_context(tc.tile_pool(name="consts", bufs=1))
        spool = ctx.enter_context(tc.tile_pool(name="state", bufs=1))
        gpool = ctx.enter_context(tc.tile_pool(name="gates", bufs=8))
        ppool = ctx.enter_context(tc.tile_pool(name="pairs", bufs=4))
        opool = ctx.enter_context(tc.tile_pool(name="outsb", bufs=3))

        xt = spool.tile([128, ncol], bf16, tag="xt")
        xch = max(1, ncol // 4)
        for c0 in range(0, ncol, xch):
            c1 = min(ncol, c0 + xch)
            nc.sync.dma_start(xt[:, c0:c1], xd[:, c0:c1])
        GRP = 4                      # supertiles per state/blend group
        ngrp = max(1, nt // GRP)
        # Per-group state tiles [128, GRP*CT]: fine deps + wide blend ops.
        sts = [spool.tile([128, GRP * CT], bf16, name=f"st{p}", tag=f"st{p}")
               for p in range(ngrp)]

        wtb = cpool.tile([128, 128], bf16, tag="wtb")
        nc.sync.dma_start(wtb[:], wtbd[:])
        wt1 = cpool.tile([128, 128], bf16, tag="wt1")
        nc.sync.dma_start(wt1[:], wt1d[:])
        a0t = cpool.tile([128, 128], bf16, tag="a0t")
        nc.sync.dma_start(a0t[:], a0td[:])
        wat = cpool.tile([128, 128], bf16, tag="wat")
        nc.sync.dma_start(wat[:], watd[:])
        bias = {}
        for k in ("br", "bz", "binn", "bhn"):
            bias[k] = cpool.tile([128, 1], f32, tag=k, name=f"b_{k}")
            nc.sync.dma_start(bias[k][:], biasd[k][:])

        # per-group z/n collection tiles so blend ops run at [128, GRP*CT]
        zp = {}
        npt = {}

        def front(it, t, psg, ps0):
            """MMs + sigmoids + STT + t2 for one supertile.

            Gate PSUM is split: G_RZ (freed fast, by the sigmoids) vs G_IH
            (freed late, by t2) so R/Z matmuls of later tiles aren't gated
            behind the slow INN/HN consumer chain."""
            first = it == 0
            wt = wt1 if first else wtb
            kk = 19 if first else 32
            p, h = t // GRP, t % GRP
            psg_rz, psg_ih = psg
            Grz = psg_rz.tile([128, 2 * CT], f32, tag="Grz",
                              name=f"Grz_{it}_{t}")
            Gih = psg_ih.tile([128, 2 * CT], f32, tag="Gih",
                              name=f"Gih_{it}_{t}")
            for g in range(4):
                out_t = Grz if g < 2 else Gih
                gc = g % 2
                for q in range(4):
                    r0 = 32 * q
                    rhs = (xt[r0:r0 + kk, t * CT:(t + 1) * CT] if first
                           else sts[p][r0:r0 + kk, h * CT:(h + 1) * CT])
                    nc.tensor.matmul(
                        out_t[r0:r0 + 32, gc * CT:(gc + 1) * CT],
                        wt[r0:r0 + kk, 32 * g:32 * g + 32],
                        rhs, start=True, stop=True,
                        tile_position=(r0, r0),
                    )
            G = None
            S0 = None
            if first:
                S0 = ps0.tile([128, CT], f32, tag="S0", name=f"S0_{t}")
                for q in range(4):
                    r0 = 32 * q
                    nc.tensor.matmul(
                        S0[r0:r0 + 32, :],
                        a0t[r0:r0 + 19, r0:r0 + 32],
                        xt[r0:r0 + 19, t * CT:(t + 1) * CT],
                        start=True, stop=True,
                        tile_position=(r0, r0),
                    )
            r = gpool.tile([128, CT], bf16, tag="r", name=f"r_{it}_{t}")
            nc.scalar.activation(r[:], Grz[:, 0:CT], AF.Sigmoid,
                                 bias=bias["br"][:])
            if h == 0:
                zp[p] = ppool.tile([128, GRP * CT], bf16, tag="zp",
                                   name=f"zp_{it}_{p}")
                npt[p] = ppool.tile([128, GRP * CT], bf16, tag="npt",
                                    name=f"np_{it}_{p}")
            nc.scalar.activation(zp[p][:, h * CT:(h + 1) * CT],
                                 Grz[:, CT:2 * CT], AF.Sigmoid,
                                 bias=bias["bz"][:])
            t1 = gpool.tile([128, CT], bf16, tag="t1", name=f"t1_{it}_{t}")
            stt_bi = nc.vector.scalar_tensor_tensor(
                t1[:], Gih[:, CT:2 * CT], bias["bhn"][:], r[:],
                ALU.add, ALU.mult)
            t2 = gpool.tile([128, CT], bf16, tag="t2", name=f"t2_{it}_{t}")
            if not first and t % 8 in (0, 3, 6):
                # rebalance: ACT evacuates INN so t2 runs in DVE 2x mode
                inn_sb = gpool.tile([128, CT], bf16, tag="innsb",
                                    name=f"innsb_{it}_{t}")
                nc.scalar.copy(inn_sb[:], Gih[:, 0:CT])
                t2_bi = nc.vector.tensor_add(t2[:], t1[:], inn_sb[:])
            else:
                t2_bi = nc.vector.tensor_add(t2[:], t1[:], Gih[:, 0:CT])
            return S0, t2, npt[p], stt_bi, t2_bi

        def tanh_op(t2, npt_t, t):
            h = t % GRP
            nc.scalar.activation(npt_t[:, h * CT:(h + 1) * CT], t2[:],
                                 AF.Tanh, bias=bias["binn"][:])

        d0g = {}

        def blend0(t, S0, zt, nt_):
            """iter-0: per-tile d (PSUM S0 read), group-wide e and h'."""
            p, h = t // GRP, t % GRP
            cs = slice(h * CT, (h + 1) * CT)
            if h == 0:
                d0g[p] = ppool.tile([128, GRP * CT], bf16, tag="dp",
                                    name=f"d0g_{p}")
            d_bi = nc.vector.tensor_sub(d0g[p][:, cs], S0[:], nt_[:, cs])
            if h == GRP - 1:
                dg = d0g.pop(p)
                e = ppool.tile([128, GRP * CT], bf16, tag="ep",
                               name=f"e0g_{p}")
                nc.vector.tensor_mul(e[:], zt[:], dg[:])
                nc.vector.tensor_add(sts[p][:], nt_[:], e[:])
            return d_bi

        bl_state = {}

        def blend_piece(it, p, k):
            """k-th piece (d/e/h') of the group blend, wide DVE ops."""
            z, n = zp_s[(it, p)], npt_s[(it, p)]
            if k == 0:
                d = ppool.tile([128, GRP * CT], bf16, tag="dp",
                               name=f"d_{it}_{p}")
                bi = nc.vector.tensor_sub(d[:], sts[p][:], n[:])
                bl_state[(it, p)] = d
            elif k == 1:
                d = bl_state[(it, p)]
                e = ppool.tile([128, GRP * CT], bf16, tag="ep",
                               name=f"e_{it}_{p}")
                bi = nc.vector.tensor_mul(e[:], z[:], d[:])
                bl_state[(it, p)] = e
            else:
                e = bl_state.pop((it, p))
                bi = nc.vector.tensor_add(sts[p][:], n[:], e[:])
                if it == LAST_IT[0]:
                    emit_output(p)
            return bi

        zp_s = {}
        npt_s = {}
        LAST_IT = [None]
        PSG_IH = [None]

        def emit_output(p):
            psg_ih = PSG_IH[0]  # actually the Grz pool (fast recycle)
            for h in range(GRP):
                t = GRP * p + h
                O = psg_ih.tile([128, CT], f32, tag="Grz", name=f"O_{t}")
                for q in range(4):
                    r0 = 32 * q
                    nc.tensor.matmul(
                        O[r0:r0 + 32, :],
                        wat[r0:r0 + 28, r0:r0 + 32],
                        sts[p][r0:r0 + 28, h * CT:(h + 1) * CT],
                        start=True, stop=True,
                        tile_position=(r0, r0),
                    )
                osb = opool.tile([128, CT], f32, tag="osb", name=f"osb_{t}")
                if h % 2 == 0:
                    nc.scalar.copy(osb[:], O[:])
                else:
                    nc.vector.tensor_copy(osb[:], O[:])
                for q in range(4):
                    nc.sync.dma_start(yd[7 * q:7 * q + 7, t * CT:(t + 1) * CT],
                                      osb[32 * q:32 * q + 7, :])

        # ---- iteration 0: per-tile skew-2 pipeline, G single-buffered
        with tc.tile_pool(name="ps0", bufs=2, space="PSUM") as ps0, \
             tc.tile_pool(name="psg1rz", bufs=1, space="PSUM") as psg1rz, \
             tc.tile_pool(name="psg1ih", bufs=2, space="PSUM") as psg1ih:
            psg1 = (psg1rz, psg1ih)
            pend = {}
            tpend = {}
            prev_d = None
            for t in range(nt):
                p = t // GRP
                S0, t2, npt_t, stt_bi, t2_bi = front(0, t, psg1, ps0)
                if prev_d is not None:
                    add_dep_helper(prev_d.ins, stt_bi.ins, sync=False,
                                   reason="pin iter0 DVE d before next STT")
                    prev_d = None
                pend[t] = (S0, zp[p], npt[p])
                tpend[t] = (t2, npt_t)
                if t >= 1:
                    tanh_op(*tpend.pop(t - 1), t - 1)
                    d_bi = blend0(t - 1, *pend.pop(t - 1))
                    add_dep_helper(t2_bi.ins, d_bi.ins, sync=False,
                                   reason="pin iter0 DVE d after t2")
                    prev_d = d_bi
            tanh_op(*tpend.pop(nt - 1), nt - 1)
            blend0(nt - 1, *pend.pop(nt - 1))

        # ---- iterations 1-6: group rounds, skew-2, G double-buffered
        # blend lag must stay < ngrp or a later iteration's matmuls would be
        # emitted (and hence ordered) before this iteration's state write
        lag = min(2, ngrp - 1) if ngrp >= 2 else 0
        with tc.tile_pool(name="psgrz", bufs=2, space="PSUM") as psgrz, \
             tc.tile_pool(name="psgih", bufs=2, space="PSUM") as psgih:
            psg = (psgrz, psgih)
            LAST_IT[0] = n_iters - 1
            PSG_IH[0] = psgrz
            rounds = [(it, p) for it in range(1, n_iters) for p in range(ngrp)]
            tq = []
            bq = []  # pending (it, p, piece) blend pieces
            prev_piece = [None]
            for R, (it, p) in enumerate(rounds):
                for h in range(GRP):
                    _, t2, npt_t, stt_bi, t2_bi = front(it, GRP * p + h,
                                                        psg, None)
                    if prev_piece[0] is not None:
                        add_dep_helper(prev_piece[0].ins, stt_bi.ins,
                                       sync=False,
                                       reason="pin DVE blend/front interleave")
                        prev_piece[0] = None
                    tq.append((t2, npt_t, GRP * p + h))
                    if len(tq) > 1:
                        tanh_op(*tq.pop(0))
                    if bq:
                        it2, p2, k2 = bq.pop(0)
                        pbi = blend_piece(it2, p2, k2)
                        if not (k2 == 2 and it2 == n_iters - 1):
                            add_dep_helper(t2_bi.ins, pbi.ins, sync=False,
                                           reason="pin DVE piece after t2")
                            prev_piece[0] = pbi
                zp_s[(it, p)], npt_s[(it, p)] = zp[p], npt[p]
                if R >= lag - 1:
                    it2, p2 = rounds[R - (lag - 1)]
                    bq.extend([(it2, p2, k) for k in range(3)])
            while tq:
                tanh_op(*tq.pop(0))
            for k in range(max(0, len(rounds) - lag + 1), len(rounds)):
                it2, p2 = rounds[k]
                bq.extend([(it2, p2, j) for j in range(3)])
            while bq:
                blend_piece(*bq.pop(0))

    nc.compile()
    return nc


_NC_CACHE = {}


def kernel(**inputs):
    x = np.asarray(inputs["x"])
    wd, ba = _host_weights(inputs)
    xs = _host_x(x)

    if "prog" not in _NC_CACHE:
        _NC_CACHE["prog"] = _build_program()
    nc = _NC_CACHE["prog"]

    in_maps = []
    for c in range(N_CORES):
        m = {"x_il": xs[c]}
        m.update({k: wd[k] for k in ("wtb", "wt1", "a0t", "wat",
                                     "br", "bz", "binn", "bhn")})
        in_maps.append(m)

    res = run_bass_kernel_spmd(nc, in_maps, core_ids=list(range(N_CORES)))
    _NC_CACHE["last_result"] = res
    outs = []
    for c in range(N_CORES):
        yc = np.asarray(res.results[c]["y"], np.float32)  # [28, NCOL]
        oc = yc.reshape(4, 7, NCOL).transpose(0, 2, 1).reshape(BC, 7)
        outs.append(oc)
    out = np.concatenate(outs, 0).reshape(B, 7, 1) + np.float32(ba)
    return out.astype(np.float32)


if __name__ == "__main__":
    rng = np.random.default_rng(0)
    demo = {"x": rng.standard_normal((B, 18), dtype=np.float32)}
    for k, shp in [("Wj", (H, 2)), ("bj", (H,)), ("Wm", (H, H)), ("bm", (H,)),
                   ("Wih_j", (3 * H, 2 * H)), ("Whh_j", (3 * H, H)),
                   ("bih_j", (3 * H,)), ("bhh_j", (3 * H,)),
                   ("Wih_m", (3 * H, H)), ("Whh_m", (3 * H, H)),
                   ("bih_m", (3 * H,)), ("bhh_m", (3 * H,)),
                   ("Wa", (1, H)), ("ba", (1,))]:
        demo[k] = (rng.standard_normal(shp) * 0.1).astype(np.float32)
    y = kernel(**demo)
    print(y.shape, y.dtype)


# revision 33
# speedup vs baseline: 1.1562x; 1.1562x over previous
"""Trainium2 Bass kernel for nn_AggregPolicy (GNN message passing / GRU chain).

Strategy:
  - Pure data parallelism: 524288 rows split across 8 cores (65536 each).
  - Feature-major on-chip layout: state s = [hj0..hj6, hm] (32 features) on
    partitions, batch on the free dim. 4 batch subgroups stacked on partitions
    (partition 32q+s) so elementwise ops use all 128 lanes.
  - Each GRU message-passing step's full linear algebra is a 32->128 linear map
    (neighbor structure folded into a banded weight matrix). Executed as 16
    small matmuls (K=32, M=32) with tile_position packing, writing gate-type-
    major PSUM banks: R | Z | INN | HN, each [128, 512].
  - Nonlinearities: ACT sigmoid/tanh with fused per-partition bias; DVE/GPSIMD
    for the remaining pointwise ops.
  - Iteration 1 consumes x directly (initial Linear layers folded into the
    first step's weights); final Linear folded into 4 output matmuls; final
    bias + layout restore on host.
"""

import sys
import numpy as np

for _p in ("/opt/trn_rl_repo",):
    if _p not in sys.path:
        sys.path.append(_p)

import ml_dtypes
from contextlib import ExitStack

import concourse.bass as bass
import concourse.bacc as bacc
import concourse.tile as tile
from concourse.tile import add_dep_helper
from concourse import mybir
from concourse.bass_utils import run_bass_kernel_spmd

BF16 = ml_dtypes.bfloat16
AF = mybir.ActivationFunctionType
ALU = mybir.AluOpType

N_CORES = 8
B = 524288
BC = B // N_CORES          # rows per core = 65536
NSUB = 4                   # batch subgroups stacked on partitions
NCOL = BC // NSUB          # free-dim columns per subgroup = 16384
CT = 512                   # columns per supertile (one PSUM bank)
NT = NCOL // CT            # 32 supertiles
H = 4
NU = 8                     # 7 joints + master
S = 32                     # state features


def _gate_blocks(p):
    """Build the 32->128 banded linear map for one message-passing step.

    Returns W (gate-major blocks) [4][32, 32] mapping state->gates and the
    four per-partition bias vectors (within one 32-wide subgroup block).
    Gate blocks: 0=R(sum), 1=Z(sum), 2=INN (input side of n), 3=HN (hidden
    side of n, bias excluded -- applied via STT scalar).
    State layout: [hj0(4) .. hj6(4), hm(4)].
    """
    Wih_j, Whh_j = p["Wih_j"], p["Whh_j"]
    Wih_m, Whh_m = p["Wih_m"], p["Whh_m"]
    W = [np.zeros((S, S), np.float64) for _ in range(4)]

    def st(u):  # state slice of unit u
        return slice(4 * u, 4 * u + 4)

    for u in range(7):
        left = None if u == 0 else st(u - 1)   # u==0 -> hm
        right = None if u == 6 else st(u + 1)  # u==6 -> zero
        for g, rows in ((0, slice(0, 4)), (1, slice(4, 8))):
            # sum gates: Wih(left,right) + Whh(self)
            Wl = Wih_j[rows, 0:4]
            Wr = Wih_j[rows, 4:8]
            Wh = Whh_j[rows, :]
            tgt = W[g][st(u), :]
            if left is None:
                tgt[:, 28:32] += Wl
            else:
                tgt[:, left] += Wl
            if right is not None:
                tgt[:, right] += Wr
            tgt[:, st(u)] += Wh
        # INN: input side only
        rows = slice(8, 12)
        tgt = W[2][st(u), :]
        if u == 0:
            tgt[:, 28:32] += Wih_j[rows, 0:4]
        else:
            tgt[:, st(u - 1)] += Wih_j[rows, 0:4]
        if u != 6:
            tgt[:, st(u + 1)] += Wih_j[rows, 4:8]
        # HN: hidden side only
        W[3][st(u), st(u)] += Whh_j[rows, :]

    # master unit (index 7, state rows 28:32); input = hj0, hidden = hm
    for g, rows in ((0, slice(0, 4)), (1, slice(4, 8))):
        W[g][28:32, 0:4] += Wih_m[rows, :]
        W[g][28:32, 28:32] += Whh_m[rows, :]
    W[2][28:32, 0:4] += Wih_m[8:12, :]
    W[3][28:32, 28:32] += Whh_m[8:12, :]

    def unit_bias(vec_j, vec_m, rows):
        b = np.zeros(S, np.float64)
        for u in range(7):
            b[st(u)] = vec_j[rows]
        b[28:32] = vec_m[rows]
        return b

    br = unit_bias(p["bih_j"], p["bih_m"], slice(0, 4)) + unit_bias(
        p["bhh_j"], p["bhh_m"], slice(0, 4))
    bz = unit_bias(p["bih_j"], p["bih_m"], slice(4, 8)) + unit_bias(
        p["bhh_j"], p["bhh_m"], slice(4, 8))
    binn = unit_bias(p["bih_j"], p["bih_m"], slice(8, 12))
    bhn = unit_bias(p["bhh_j"], p["bhh_m"], slice(8, 12))
    return W, (br, bz, binn, bhn)


def _a0_ext(p):
    """[32, 19] initial-linear map: state0 = A0e @ [x(18); 1]."""
    A = np.zeros((S, 19), np.float64)
    Wj, bj, Wm, bm = p["Wj"], p["bj"], p["Wm"], p["bm"]
    for u in range(7):
        A[4 * u:4 * u + 4, 4 + u] = Wj[:, 0]
        A[4 * u:4 * u + 4, 11 + u] = Wj[:, 1]
        A[4 * u:4 * u + 4, 18] = bj
    A[28:32, 0:4] = Wm
    A[28:32, 18] = bm
    return A


def _host_weights(inputs):
    p = {k: np.asarray(v, np.float64) for k, v in inputs.items() if k != "x"}
    W, (br, bz, binn, bhn) = _gate_blocks(p)
    A0e = _a0_ext(p)

    # wtb [128,128]: rows 32q+k (k<32) = state idx, cols 32g+m = gate out m of block g
    wtb = np.zeros((128, 128), np.float64)
    # wt1 [128,128]: iteration-1 gate weights consuming xe(19) directly
    wt1 = np.zeros((128, 128), np.float64)
    # a0t: diag blocks for S0 psum (iter-1 blend h operand)
    a0t = np.zeros((128, 128), np.float64)
    # wat: diag blocks for output linear (state -> 7 activations)
    wat = np.zeros((128, 128), np.float64)
    Wa = p["Wa"]  # [1, 4]
    for q in range(4):
        r0 = 32 * q
        for g in range(4):
            wtb[r0:r0 + 32, 32 * g:32 * g + 32] = W[g].T
            W1g = W[g] @ A0e  # [32, 19]
            wt1[r0:r0 + 19, 32 * g:32 * g + 32] = W1g.T
        a0t[r0:r0 + 19, r0:r0 + 32] = A0e.T
        for u in range(7):
            wat[r0 + 4 * u:r0 + 4 * u + 4, r0 + u] = Wa[0, :]

    def bias128(v):
        return np.tile(v, 4).astype(np.float32).reshape(128, 1)

    return {
        "wtb": wtb.astype(BF16), "wt1": wt1.astype(BF16),
        "a0t": a0t.astype(BF16), "wat": wat.astype(BF16),
        "br": bias128(br), "bz": bias128(bz),
        "binn": bias128(binn), "bhn": bias128(bhn),
    }, float(np.asarray(inputs["ba"]).reshape(-1)[0])


def _host_x(x):
    """x [B,18] fp32 -> per-core [128, NCOL] bf16 (partition 32q+k, k<19)."""
    xs = []
    for c in range(N_CORES):
        xc = np.asarray(x[c * BC:(c + 1) * BC], np.float32)
        arr = np.zeros((4, 32, NCOL), np.float32)
        arr[:, 0:18, :] = xc.reshape(4, NCOL, 18).transpose(0, 2, 1)
        arr[:, 18, :] = 1.0
        xs.append(arr.reshape(128, NCOL).astype(BF16))
    return xs


def _build_program(ncol=NCOL, nt=NT, n_iters=7):
    nc = bacc.Bacc("TRN2", target_bir_lowering=False, debug=False,
                   num_devices=N_CORES)
    f32 = mybir.dt.float32
    bf16 = mybir.dt.bfloat16

    xd = nc.dram_tensor("x_il", [128, ncol], bf16, kind="ExternalInput").ap()
    wtbd = nc.dram_tensor("wtb", [128, 128], bf16, kind="ExternalInput").ap()
    wt1d = nc.dram_tensor("wt1", [128, 128], bf16, kind="ExternalInput").ap()
    a0td = nc.dram_tensor("a0t", [128, 128], bf16, kind="ExternalInput").ap()
    watd = nc.dram_tensor("wat", [128, 128], bf16, kind="ExternalInput").ap()
    biasd = {k: nc.dram_tensor(k, [128, 1], f32, kind="ExternalInput").ap()
             for k in ("br", "bz", "binn", "bhn")}
    yd = nc.dram_tensor("y", [28, ncol], f32, kind="ExternalOutput").ap()

    with tile.TileContext(nc) as tc, ExitStack() as ctx:
        cpool = ctx.enter_context(tc.tile_pool(name="consts", bufs=1))
        spool = ctx.enter_context(tc.tile_pool(name="state", bufs=1))
        gpool = ctx.enter_context(tc.tile_pool(name="gates", bufs=8))
        ppool = ctx.enter_context(tc.tile_pool(name="pairs", bufs=4))
        opool = ctx.enter_context(tc.tile_pool(name="outsb", bufs=3))

        xt = spool.tile([128, ncol], bf16, tag="xt")
        xch = max(1, ncol // 4)
        for c0 in range(0, ncol, xch):
            c1 = min(ncol, c0 + xch)
            nc.sync.dma_start(xt[:, c0:c1], xd[:, c0:c1])
        GRP = 4                      # supertiles per state/blend group
        ngrp = max(1, nt // GRP)
        # Per-group state tiles [128, GRP*CT]: fine deps + wide blend ops.
        sts = [spool.tile([128, GRP * CT], bf16, name=f"st{p}", tag=f"st{p}")
               for p in range(ngrp)]

        wtb = cpool.tile([128, 128], bf16, tag="wtb")
        nc.sync.dma_start(wtb[:], wtbd[:])
        wt1 = cpool.tile([128, 128], bf16, tag="wt1")
        nc.sync.dma_start(wt1[:], wt1d[:])
        a0t = cpool.tile([128, 128], bf16, tag="a0t")
        nc.sync.dma_start(a0t[:], a0td[:])
        wat = cpool.tile([128, 128], bf16, tag="wat")
        nc.sync.dma_start(wat[:], watd[:])
        bias = {}
        for k in ("br", "bz", "binn", "bhn"):
            bias[k] = cpool.tile([128, 1], f32, tag=k, name=f"b_{k}")
            nc.sync.dma_start(bias[k][:], biasd[k][:])

        # per-group z/n collection tiles so blend ops run at [128, GRP*CT]
        zp = {}
        npt = {}

        def front(it, t, psg, ps0):
            """MMs + sigmoids + STT + t2 for one supertile.

            Gate PSUM is split: G_RZ (freed fast, by the sigmoids) vs G_IH
            (freed late, by t2) so R/Z matmuls of later tiles aren't gated
            behind the slow INN/HN consumer chain."""
            first = it == 0
            wt = wt1 if first else wtb
            kk = 19 if first else 32
            p, h = t // GRP, t % GRP
            psg_rz, psg_ih = psg
            Grz = psg_rz.tile([128, 2 * CT], f32, tag="Grz",
                              name=f"Grz_{it}_{t}")
            Gih = psg_ih.tile([128, 2 * CT], f32, tag="Gih",
                              name=f"Gih_{it}_{t}")
            for g in range(4):
                out_t = Grz if g < 2 else Gih
                gc = g % 2
                for q in range(4):
                    r0 = 32 * q
                    rhs = (xt[r0:r0 + kk, t * CT:(t + 1) * CT] if first
                           else sts[p][r0:r0 + kk, h * CT:(h + 1) * CT])
                    nc.tensor.matmul(
                        out_t[r0:r0 + 32, gc * CT:(gc + 1) * CT],
                        wt[r0:r0 + kk, 32 * g:32 * g + 32],
                        rhs, start=True, stop=True,
                        tile_position=(r0, r0),
                    )
            G = None
            S0 = None
            if first:
                S0 = ps0.tile([128, CT], f32, tag="S0", name=f"S0_{t}")
                for q in range(4):
                    r0 = 32 * q
                    nc.tensor.matmul(
                        S0[r0:r0 + 32, :],
                        a0t[r0:r0 + 19, r0:r0 + 32],
                        xt[r0:r0 + 19, t * CT:(t + 1) * CT],
                        start=True, stop=True,
                        tile_position=(r0, r0),
                    )
            r = gpool.tile([128, CT], bf16, tag="r", name=f"r_{it}_{t}")
            nc.scalar.activation(r[:], Grz[:, 0:CT], AF.Sigmoid,
                                 bias=bias["br"][:])
            if h == 0:
                zp[p] = ppool.tile([128, GRP * CT], bf16, tag="zp",
                                   name=f"zp_{it}_{p}")
                npt[p] = ppool.tile([128, GRP * CT], bf16, tag="npt",
                                    name=f"np_{it}_{p}")
            nc.scalar.activation(zp[p][:, h * CT:(h + 1) * CT],
                                 Grz[:, CT:2 * CT], AF.Sigmoid,
                                 bias=bias["bz"][:])
            t1 = gpool.tile([128, CT], bf16, tag="t1", name=f"t1_{it}_{t}")
            t2 = gpool.tile([128, CT], bf16, tag="t2", name=f"t2_{it}_{t}")
            if not first and t % 8 < 2:
                # full evacuation: one wide ACT copy frees Gih right after
                # the matmuls (no r-gating) and STT+t2 run in DVE 2x mode
                ih_sb = gpool.tile([128, 2 * CT], bf16, tag="ihsb",
                                   name=f"ihsb_{it}_{t}")
                nc.scalar.copy(ih_sb[:], Gih[:])
                stt_bi = nc.vector.scalar_tensor_tensor(
                    t1[:], ih_sb[:, CT:2 * CT], bias["bhn"][:], r[:],
                    ALU.add, ALU.mult)
                t2_bi = nc.vector.tensor_add(t2[:], t1[:], ih_sb[:, 0:CT])
            else:
                stt_bi = nc.vector.scalar_tensor_tensor(
                    t1[:], Gih[:, CT:2 * CT], bias["bhn"][:], r[:],
                    ALU.add, ALU.mult)
                t2_bi = nc.vector.tensor_add(t2[:], t1[:], Gih[:, 0:CT])
            return S0, t2, npt[p], stt_bi, t2_bi

        def tanh_op(t2, npt_t, t):
            h = t % GRP
            nc.scalar.activation(npt_t[:, h * CT:(h + 1) * CT], t2[:],
                                 AF.Tanh, bias=bias["binn"][:])

        d0g = {}

        def blend0(t, S0, zt, nt_):
            """iter-0: per-tile d (PSUM S0 read), group-wide e and h'."""
            p, h = t // GRP, t % GRP
            cs = slice(h * CT, (h + 1) * CT)
            if h == 0:
                d0g[p] = ppool.tile([128, GRP * CT], bf16, tag="dp",
                                    name=f"d0g_{p}")
            nc.vector.tensor_sub(d0g[p][:, cs], S0[:], nt_[:, cs])
            if h == GRP - 1:
                dg = d0g.pop(p)
                e = ppool.tile([128, GRP * CT], bf16, tag="ep",
                               name=f"e0g_{p}")
                nc.vector.tensor_mul(e[:], zt[:], dg[:])
                nc.vector.tensor_add(sts[p][:], nt_[:], e[:])

        bl_state = {}

        def blend_piece(it, p, k):
            """k-th piece (d/e/h') of the group blend, wide DVE ops."""
            z, n = zp_s[(it, p)], npt_s[(it, p)]
            if k == 0:
                d = ppool.tile([128, GRP * CT], bf16, tag="dp",
                               name=f"d_{it}_{p}")
                bi = nc.vector.tensor_sub(d[:], sts[p][:], n[:])
                bl_state[(it, p)] = d
            elif k == 1:
                d = bl_state[(it, p)]
                e = ppool.tile([128, GRP * CT], bf16, tag="ep",
                               name=f"e_{it}_{p}")
                bi = nc.vector.tensor_mul(e[:], z[:], d[:])
                bl_state[(it, p)] = e
            else:
                e = bl_state.pop((it, p))
                bi = nc.vector.tensor_add(sts[p][:], n[:], e[:])
                if it == LAST_IT[0]:
                    emit_output(p)
            return bi

        zp_s = {}
        npt_s = {}
        LAST_IT = [None]
        PSG_IH = [None]

        def emit_output(p):
            psg_ih = PSG_IH[0]  # actually the Grz pool (fast recycle)
            for h in range(GRP):
                t = GRP * p + h
                O = psg_ih.tile([128, CT], f32, tag="Grz", name=f"O_{t}")
                for q in range(4):
                    r0 = 32 * q
                    nc.tensor.matmul(
                        O[r0:r0 + 32, :],
                        wat[r0:r0 + 28, r0:r0 + 32],
                        sts[p][r0:r0 + 28, h * CT:(h + 1) * CT],
                        start=True, stop=True,
                        tile_position=(r0, r0),
                    )
                osb = opool.tile([128, CT], f32, tag="osb", name=f"osb_{t}")
                if h % 2 == 0:
                    nc.scalar.copy(osb[:], O[:])
                else:
                    nc.vector.tensor_copy(osb[:], O[:])
                for q in range(4):
                    nc.sync.dma_start(yd[7 * q:7 * q + 7, t * CT:(t + 1) * CT],
                                      osb[32 * q:32 * q + 7, :])

        # ---- iteration 0: per-tile skew-2 pipeline, G single-buffered
        with tc.tile_pool(name="ps0", bufs=2, space="PSUM") as ps0, \
             tc.tile_pool(name="psg1rz", bufs=1, space="PSUM") as psg1rz, \
             tc.tile_pool(name="psg1ih", bufs=2, space="PSUM") as psg1ih:
            psg1 = (psg1rz, psg1ih)
            pend = {}
            tpend = {}
            for t in range(nt):
                p = t // GRP
                S0, t2, npt_t, _, _ = front(0, t, psg1, ps0)
                pend[t] = (S0, zp[p], npt[p])
                tpend[t] = (t2, npt_t)
                if t >= 1:
                    tanh_op(*tpend.pop(t - 1), t - 1)
                    blend0(t - 1, *pend.pop(t - 1))
            tanh_op(*tpend.pop(nt - 1), nt - 1)
            blend0(nt - 1, *pend.pop(nt - 1))

        # ---- iterations 1-6: group rounds, skew-2, G double-buffered
        # blend lag must stay < ngrp or a later iteration's matmuls would be
        # emitted (and hence ordered) before this iteration's state write
        lag = min(2, ngrp - 1) if ngrp >= 2 else 0
        with tc.tile_pool(name="psgrz", bufs=2, space="PSUM") as psgrz, \
             tc.tile_pool(name="psgih", bufs=2, space="PSUM") as psgih:
            psg = (psgrz, psgih)
            LAST_IT[0] = n_iters - 1
            PSG_IH[0] = psgrz
            rounds = [(it, p) for it in range(1, n_iters) for p in range(ngrp)]
            tq = []
            bq = []  # pending (it, p, piece) blend pieces
            prev_piece = [None]
            for R, (it, p) in enumerate(rounds):
                for h in range(GRP):
                    _, t2, npt_t, stt_bi, t2_bi = front(it, GRP * p + h,
                                                        psg, None)
                    if prev_piece[0] is not None:
                        add_dep_helper(prev_piece[0].ins, stt_bi.ins,
                                       sync=False,
                                       reason="pin DVE blend/front interleave")
                        prev_piece[0] = None
                    tq.append((t2, npt_t, GRP * p + h))
                    if len(tq) > 1:
                        tanh_op(*tq.pop(0))
                    if bq:
                        it2, p2, k2 = bq.pop(0)
                        pbi = blend_piece(it2, p2, k2)
                        if not (k2 == 2 and it2 == n_iters - 1):
                            add_dep_helper(t2_bi.ins, pbi.ins, sync=False,
                                           reason="pin DVE piece after t2")
                            prev_piece[0] = pbi
                zp_s[(it, p)], npt_s[(it, p)] = zp[p], npt[p]
                if R >= lag - 1:
                    it2, p2 = rounds[R - (lag - 1)]
                    bq.extend([(it2, p2, k) for k in range(3)])
            while tq:
                tanh_op(*tq.pop(0))
            for k in range(max(0, len(rounds) - lag + 1), len(rounds)):
                it2, p2 = rounds[k]
                bq.extend([(it2, p2, j) for j in range(3)])
            while bq:
                blend_piece(*bq.pop(0))

    nc.compile()
    return nc


_NC_CACHE = {}


def kernel(**inputs):
    x = np.asarray(inputs["x"])
    wd, ba = _host_weights(inputs)
    xs = _host_x(x)

    if "prog" not in _NC_CACHE:
        _NC_CACHE["prog"] = _build_program()
    nc = _NC_CACHE["prog"]

    in_maps = []
    for c in range(N_CORES):
        m = {"x_il": xs[c]}
        m.update({k: wd[k] for k in ("wtb", "wt1", "a0t", "wat",
                                     "br", "bz", "binn", "bhn")})
        in_maps.append(m)

    res = run_bass_kernel_spmd(nc, in_maps, core_ids=list(range(N_CORES)))
    _NC_CACHE["last_result"] = res
    outs = []
    for c in range(N_CORES):
        yc = np.asarray(res.results[c]["y"], np.float32)  # [28, NCOL]
        oc = yc.reshape(4, 7, NCOL).transpose(0, 2, 1).reshape(BC, 7)
        outs.append(oc)
    out = np.concatenate(outs, 0).reshape(B, 7, 1) + np.float32(ba)
    return out.astype(np.float32)


if __name__ == "__main__":
    rng = np.random.default_rng(0)
    demo = {"x": rng.standard_normal((B, 18), dtype=np.float32)}
    for k, shp in [("Wj", (H, 2)), ("bj", (H,)), ("Wm", (H, H)), ("bm", (H,)),
                   ("Wih_j", (3 * H, 2 * H)), ("Whh_j", (3 * H, H)),
                   ("bih_j", (3 * H,)), ("bhh_j", (3 * H,)),
                   ("Wih_m", (3 * H, H)), ("Whh_m", (3 * H, H)),
                   ("bih_m", (3 * H,)), ("bhh_m", (3 * H,)),
                   ("Wa", (1, H)), ("ba", (1,))]:
        demo[k] = (rng.standard_normal(shp) * 0.1).astype(np.float32)
    y = kernel(**demo)
    print(y.shape, y.dtype)


# revision 35
# speedup vs baseline: 1.2142x; 1.0502x over previous
"""Trainium2 Bass kernel for nn_AggregPolicy (GNN message passing / GRU chain).

Strategy:
  - Pure data parallelism: 524288 rows split across 8 cores (65536 each).
  - Feature-major on-chip layout: state s = [hj0..hj6, hm] (32 features) on
    partitions, batch on the free dim. 4 batch subgroups stacked on partitions
    (partition 32q+s) so elementwise ops use all 128 lanes.
  - Each GRU message-passing step's full linear algebra is a 32->128 linear map
    (neighbor structure folded into a banded weight matrix). Executed as 16
    small matmuls (K=32, M=32) with tile_position packing, writing gate-type-
    major PSUM banks: R | Z | INN | HN, each [128, 512].
  - Nonlinearities: ACT sigmoid/tanh with fused per-partition bias; DVE/GPSIMD
    for the remaining pointwise ops.
  - Iteration 1 consumes x directly (initial Linear layers folded into the
    first step's weights); final Linear folded into 4 output matmuls; final
    bias + layout restore on host.
"""

import sys
import numpy as np

for _p in ("/opt/trn_rl_repo",):
    if _p not in sys.path:
        sys.path.append(_p)

import ml_dtypes
from contextlib import ExitStack

import concourse.bass as bass
import concourse.bacc as bacc
import concourse.tile as tile
from concourse.tile import add_dep_helper
from concourse import mybir
from concourse.bass_utils import run_bass_kernel_spmd

BF16 = ml_dtypes.bfloat16
AF = mybir.ActivationFunctionType
ALU = mybir.AluOpType

N_CORES = 8
B = 524288
BC = B // N_CORES          # rows per core = 65536
NSUB = 4                   # batch subgroups stacked on partitions
NCOL = BC // NSUB          # free-dim columns per subgroup = 16384
CT = 512                   # columns per supertile (one PSUM bank)
NT = NCOL // CT            # 32 supertiles
H = 4
NU = 8                     # 7 joints + master
S = 32                     # state features


def _gate_blocks(p):
    """Build the 32->128 banded linear map for one message-passing step.

    Returns W (gate-major blocks) [4][32, 32] mapping state->gates and the
    four per-partition bias vectors (within one 32-wide subgroup block).
    Gate blocks: 0=R(sum), 1=Z(sum), 2=INN (input side of n), 3=HN (hidden
    side of n, bias excluded -- applied via STT scalar).
    State layout: [hj0(4) .. hj6(4), hm(4)].
    """
    Wih_j, Whh_j = p["Wih_j"], p["Whh_j"]
    Wih_m, Whh_m = p["Wih_m"], p["Whh_m"]
    W = [np.zeros((S, S), np.float64) for _ in range(4)]

    def st(u):  # state slice of unit u
        return slice(4 * u, 4 * u + 4)

    for u in range(7):
        left = None if u == 0 else st(u - 1)   # u==0 -> hm
        right = None if u == 6 else st(u + 1)  # u==6 -> zero
        for g, rows in ((0, slice(0, 4)), (1, slice(4, 8))):
            # sum gates: Wih(left,right) + Whh(self)
            Wl = Wih_j[rows, 0:4]
            Wr = Wih_j[rows, 4:8]
            Wh = Whh_j[rows, :]
            tgt = W[g][st(u), :]
            if left is None:
                tgt[:, 28:32] += Wl
            else:
                tgt[:, left] += Wl
            if right is not None:
                tgt[:, right] += Wr
            tgt[:, st(u)] += Wh
        # INN: input side only
        rows = slice(8, 12)
        tgt = W[2][st(u), :]
        if u == 0:
            tgt[:, 28:32] += Wih_j[rows, 0:4]
        else:
            tgt[:, st(u - 1)] += Wih_j[rows, 0:4]
        if u != 6:
            tgt[:, st(u + 1)] += Wih_j[rows, 4:8]
        # HN: hidden side only
        W[3][st(u), st(u)] += Whh_j[rows, :]

    # master unit (index 7, state rows 28:32); input = hj0, hidden = hm
    for g, rows in ((0, slice(0, 4)), (1, slice(4, 8))):
        W[g][28:32, 0:4] += Wih_m[rows, :]
        W[g][28:32, 28:32] += Whh_m[rows, :]
    W[2][28:32, 0:4] += Wih_m[8:12, :]
    W[3][28:32, 28:32] += Whh_m[8:12, :]

    def unit_bias(vec_j, vec_m, rows):
        b = np.zeros(S, np.float64)
        for u in range(7):
            b[st(u)] = vec_j[rows]
        b[28:32] = vec_m[rows]
        return b

    br = unit_bias(p["bih_j"], p["bih_m"], slice(0, 4)) + unit_bias(
        p["bhh_j"], p["bhh_m"], slice(0, 4))
    bz = unit_bias(p["bih_j"], p["bih_m"], slice(4, 8)) + unit_bias(
        p["bhh_j"], p["bhh_m"], slice(4, 8))
    binn = unit_bias(p["bih_j"], p["bih_m"], slice(8, 12))
    bhn = unit_bias(p["bhh_j"], p["bhh_m"], slice(8, 12))
    return W, (br, bz, binn, bhn)


def _a0_ext(p):
    """[32, 19] initial-linear map: state0 = A0e @ [x(18); 1]."""
    A = np.zeros((S, 19), np.float64)
    Wj, bj, Wm, bm = p["Wj"], p["bj"], p["Wm"], p["bm"]
    for u in range(7):
        A[4 * u:4 * u + 4, 4 + u] = Wj[:, 0]
        A[4 * u:4 * u + 4, 11 + u] = Wj[:, 1]
        A[4 * u:4 * u + 4, 18] = bj
    A[28:32, 0:4] = Wm
    A[28:32, 18] = bm
    return A


def _host_weights(inputs):
    p = {k: np.asarray(v, np.float64) for k, v in inputs.items() if k != "x"}
    W, (br, bz, binn, bhn) = _gate_blocks(p)
    A0e = _a0_ext(p)

    # wtb [128,128]: rows 32q+k (k<32) = state idx, cols 32g+m = gate out m of block g
    wtb = np.zeros((128, 128), np.float64)
    # wt1 [128,128]: iteration-1 gate weights consuming xe(19) directly
    wt1 = np.zeros((128, 128), np.float64)
    # a0t: diag blocks for S0 psum (iter-1 blend h operand)
    a0t = np.zeros((128, 128), np.float64)
    # wat: diag blocks for output linear (state -> 7 activations)
    wat = np.zeros((128, 128), np.float64)
    Wa = p["Wa"]  # [1, 4]
    for q in range(4):
        r0 = 32 * q
        for g in range(4):
            wtb[r0:r0 + 32, 32 * g:32 * g + 32] = W[g].T
            W1g = W[g] @ A0e  # [32, 19]
            wt1[r0:r0 + 19, 32 * g:32 * g + 32] = W1g.T
        a0t[r0:r0 + 19, r0:r0 + 32] = A0e.T
        for u in range(7):
            wat[r0 + 4 * u:r0 + 4 * u + 4, r0 + u] = Wa[0, :]

    def bias128(v):
        return np.tile(v, 4).astype(np.float32).reshape(128, 1)

    return {
        "wtb": wtb.astype(BF16), "wt1": wt1.astype(BF16),
        "a0t": a0t.astype(BF16), "wat": wat.astype(BF16),
        "br": bias128(br), "bz": bias128(bz),
        "binn": bias128(binn), "bhn": bias128(bhn),
    }, float(np.asarray(inputs["ba"]).reshape(-1)[0])


def _host_x(x):
    """x [B,18] fp32 -> per-core [128, NCOL] bf16 (partition 32q+k, k<19)."""
    xs = []
    for c in range(N_CORES):
        xc = np.asarray(x[c * BC:(c + 1) * BC], np.float32)
        arr = np.zeros((4, 32, NCOL), np.float32)
        arr[:, 0:18, :] = xc.reshape(4, NCOL, 18).transpose(0, 2, 1)
        arr[:, 18, :] = 1.0
        xs.append(arr.reshape(128, NCOL).astype(BF16))
    return xs


def _build_program(ncol=NCOL, nt=NT, n_iters=7):
    nc = bacc.Bacc("TRN2", target_bir_lowering=False, debug=False,
                   num_devices=N_CORES)
    f32 = mybir.dt.float32
    bf16 = mybir.dt.bfloat16

    xd = nc.dram_tensor("x_il", [128, ncol], bf16, kind="ExternalInput").ap()
    wtbd = nc.dram_tensor("wtb", [128, 128], bf16, kind="ExternalInput").ap()
    wt1d = nc.dram_tensor("wt1", [128, 128], bf16, kind="ExternalInput").ap()
    a0td = nc.dram_tensor("a0t", [128, 128], bf16, kind="ExternalInput").ap()
    watd = nc.dram_tensor("wat", [128, 128], bf16, kind="ExternalInput").ap()
    biasd = {k: nc.dram_tensor(k, [128, 1], f32, kind="ExternalInput").ap()
             for k in ("br", "bz", "binn", "bhn")}
    yd = nc.dram_tensor("y", [28, ncol], f32, kind="ExternalOutput").ap()

    with tile.TileContext(nc) as tc, ExitStack() as ctx:
        cpool = ctx.enter_context(tc.tile_pool(name="consts", bufs=1))
        spool = ctx.enter_context(tc.tile_pool(name="state", bufs=1))
        gpool = ctx.enter_context(tc.tile_pool(name="gates", bufs=8))
        ppool = ctx.enter_context(tc.tile_pool(name="pairs", bufs=4))
        opool = ctx.enter_context(tc.tile_pool(name="outsb", bufs=3))

        xch = max(CT, ncol // 4)
        xts = []
        for ci, c0 in enumerate(range(0, ncol, xch)):
            c1 = min(ncol, c0 + xch)
            xt_c = spool.tile([128, c1 - c0], bf16, name=f"xt{ci}",
                              tag=f"xt{ci}")
            nc.sync.dma_start(xt_c[:], xd[:, c0:c1])
            xts.append((c0, xt_c))

        def x_slice(r0, kk, t):
            c = t * CT
            for c0, xt_c in xts:
                if c0 <= c < c0 + xt_c.shape[1]:
                    return xt_c[r0:r0 + kk, c - c0:c - c0 + CT]
            raise AssertionError
        GRP = 4                      # supertiles per state/blend group
        ngrp = max(1, nt // GRP)
        # Per-group state tiles [128, GRP*CT]: fine deps + wide blend ops.
        sts = [spool.tile([128, GRP * CT], bf16, name=f"st{p}", tag=f"st{p}")
               for p in range(ngrp)]

        wtb = cpool.tile([128, 128], bf16, tag="wtb")
        nc.sync.dma_start(wtb[:], wtbd[:])
        wt1 = cpool.tile([128, 128], bf16, tag="wt1")
        nc.sync.dma_start(wt1[:], wt1d[:])
        a0t = cpool.tile([128, 128], bf16, tag="a0t")
        nc.sync.dma_start(a0t[:], a0td[:])
        wat = cpool.tile([128, 128], bf16, tag="wat")
        nc.sync.dma_start(wat[:], watd[:])
        bias = {}
        for k in ("br", "bz", "binn", "bhn"):
            bias[k] = cpool.tile([128, 1], f32, tag=k, name=f"b_{k}")
            nc.sync.dma_start(bias[k][:], biasd[k][:])

        # per-group z/n collection tiles so blend ops run at [128, GRP*CT]
        zp = {}
        npt = {}

        def front(it, t, psg, ps0):
            """MMs + sigmoids + STT + t2 for one supertile.

            Gate PSUM is split: G_RZ (freed fast, by the sigmoids) vs G_IH
            (freed late, by t2) so R/Z matmuls of later tiles aren't gated
            behind the slow INN/HN consumer chain."""
            first = it == 0
            wt = wt1 if first else wtb
            kk = 19 if first else 32
            p, h = t // GRP, t % GRP
            psg_rz, psg_ih = psg
            Grz = psg_rz.tile([128, 2 * CT], f32, tag="Grz",
                              name=f"Grz_{it}_{t}")
            Gih = psg_ih.tile([128, 2 * CT], f32, tag="Gih",
                              name=f"Gih_{it}_{t}")
            for g in range(4):
                out_t = Grz if g < 2 else Gih
                gc = g % 2
                for q in range(4):
                    r0 = 32 * q
                    rhs = (x_slice(r0, kk, t) if first
                           else sts[p][r0:r0 + kk, h * CT:(h + 1) * CT])
                    nc.tensor.matmul(
                        out_t[r0:r0 + 32, gc * CT:(gc + 1) * CT],
                        wt[r0:r0 + kk, 32 * g:32 * g + 32],
                        rhs, start=True, stop=True,
                        tile_position=(r0, r0),
                    )
            G = None
            S0 = None
            if first:
                S0 = ps0.tile([128, CT], f32, tag="S0", name=f"S0_{t}")
                for q in range(4):
                    r0 = 32 * q
                    nc.tensor.matmul(
                        S0[r0:r0 + 32, :],
                        a0t[r0:r0 + 19, r0:r0 + 32],
                        x_slice(r0, 19, t),
                        start=True, stop=True,
                        tile_position=(r0, r0),
                    )
            r = gpool.tile([128, CT], bf16, tag="r", name=f"r_{it}_{t}")
            nc.scalar.activation(r[:], Grz[:, 0:CT], AF.Sigmoid,
                                 bias=bias["br"][:])
            if h == 0:
                zp[p] = ppool.tile([128, GRP * CT], bf16, tag="zp",
                                   name=f"zp_{it}_{p}")
                npt[p] = ppool.tile([128, GRP * CT], bf16, tag="npt",
                                    name=f"np_{it}_{p}")
            nc.scalar.activation(zp[p][:, h * CT:(h + 1) * CT],
                                 Grz[:, CT:2 * CT], AF.Sigmoid,
                                 bias=bias["bz"][:])
            t1 = gpool.tile([128, CT], bf16, tag="t1", name=f"t1_{it}_{t}")
            stt_bi = nc.vector.scalar_tensor_tensor(
                t1[:], Gih[:, CT:2 * CT], bias["bhn"][:], r[:],
                ALU.add, ALU.mult)
            t2 = gpool.tile([128, CT], bf16, tag="t2", name=f"t2_{it}_{t}")
            if not first and t % 8 < 3:
                # rebalance: ACT evacuates INN so t2 runs in DVE 2x mode
                inn_sb = gpool.tile([128, CT], bf16, tag="innsb",
                                    name=f"innsb_{it}_{t}")
                nc.scalar.copy(inn_sb[:], Gih[:, 0:CT])
                t2_bi = nc.vector.tensor_add(t2[:], t1[:], inn_sb[:])
            else:
                t2_bi = nc.vector.tensor_add(t2[:], t1[:], Gih[:, 0:CT])
            return S0, t2, npt[p], stt_bi, t2_bi

        def tanh_op(t2, npt_t, t):
            h = t % GRP
            nc.scalar.activation(npt_t[:, h * CT:(h + 1) * CT], t2[:],
                                 AF.Tanh, bias=bias["binn"][:])

        d0g = {}

        def blend0(t, S0, zt, nt_):
            """iter-0: per-tile d (PSUM S0 read), group-wide e and h'."""
            p, h = t // GRP, t % GRP
            cs = slice(h * CT, (h + 1) * CT)
            if h == 0:
                d0g[p] = ppool.tile([128, GRP * CT], bf16, tag="dp",
                                    name=f"d0g_{p}")
            nc.vector.tensor_sub(d0g[p][:, cs], S0[:], nt_[:, cs])
            if h == GRP - 1:
                dg = d0g.pop(p)
                e = ppool.tile([128, GRP * CT], bf16, tag="ep",
                               name=f"e0g_{p}")
                nc.vector.tensor_mul(e[:], zt[:], dg[:])
                nc.vector.tensor_add(sts[p][:], nt_[:], e[:])

        bl_state = {}

        def blend_piece(it, p, k):
            """k-th piece (d/e/h') of the group blend, wide DVE ops."""
            z, n = zp_s[(it, p)], npt_s[(it, p)]
            if k == 0:
                d = ppool.tile([128, GRP * CT], bf16, tag="dp",
                               name=f"d_{it}_{p}")
                bi = nc.vector.tensor_sub(d[:], sts[p][:], n[:])
                bl_state[(it, p)] = d
            elif k == 1:
                d = bl_state[(it, p)]
                e = ppool.tile([128, GRP * CT], bf16, tag="ep",
                               name=f"e_{it}_{p}")
                bi = nc.vector.tensor_mul(e[:], z[:], d[:])
                bl_state[(it, p)] = e
            else:
                e = bl_state.pop((it, p))
                bi = nc.vector.tensor_add(sts[p][:], n[:], e[:])
                if it == LAST_IT[0]:
                    emit_output(p)
            return bi

        zp_s = {}
        npt_s = {}
        LAST_IT = [None]
        PSG_IH = [None]

        def emit_output(p):
            psg_ih = PSG_IH[0]  # actually the Grz pool (fast recycle)
            for h in range(GRP):
                t = GRP * p + h
                O = psg_ih.tile([128, CT], f32, tag="Grz", name=f"O_{t}")
                for q in range(4):
                    r0 = 32 * q
                    nc.tensor.matmul(
                        O[r0:r0 + 32, :],
                        wat[r0:r0 + 28, r0:r0 + 32],
                        sts[p][r0:r0 + 28, h * CT:(h + 1) * CT],
                        start=True, stop=True,
                        tile_position=(r0, r0),
                    )
                osb = opool.tile([128, CT], f32, tag="osb", name=f"osb_{t}")
                if h % 2 == 0:
                    nc.scalar.copy(osb[:], O[:])
                else:
                    nc.vector.tensor_copy(osb[:], O[:])
                for q in range(4):
                    nc.sync.dma_start(yd[7 * q:7 * q + 7, t * CT:(t + 1) * CT],
                                      osb[32 * q:32 * q + 7, :])

        # ---- iteration 0: per-tile skew-2 pipeline, G single-buffered
        with tc.tile_pool(name="ps0", bufs=2, space="PSUM") as ps0, \
             tc.tile_pool(name="psg1rz", bufs=1, space="PSUM") as psg1rz, \
             tc.tile_pool(name="psg1ih", bufs=2, space="PSUM") as psg1ih:
            psg1 = (psg1rz, psg1ih)
            pend = {}
            tpend = {}
            for t in range(nt):
                p = t // GRP
                S0, t2, npt_t, _, _ = front(0, t, psg1, ps0)
                pend[t] = (S0, zp[p], npt[p])
                tpend[t] = (t2, npt_t)
                if t >= 1:
                    tanh_op(*tpend.pop(t - 1), t - 1)
                    blend0(t - 1, *pend.pop(t - 1))
            tanh_op(*tpend.pop(nt - 1), nt - 1)
            blend0(nt - 1, *pend.pop(nt - 1))

        # ---- iterations 1-6: group rounds, skew-2, G double-buffered
        # blend lag must stay < ngrp or a later iteration's matmuls would be
        # emitted (and hence ordered) before this iteration's state write
        lag = min(2, ngrp - 1) if ngrp >= 2 else 0
        with tc.tile_pool(name="psgrz", bufs=2, space="PSUM") as psgrz, \
             tc.tile_pool(name="psgih", bufs=2, space="PSUM") as psgih:
            psg = (psgrz, psgih)
            LAST_IT[0] = n_iters - 1
            PSG_IH[0] = psgrz
            rounds = [(it, p) for it in range(1, n_iters) for p in range(ngrp)]
            tq = []
            bq = []  # pending (it, p, piece) blend pieces
            prev_piece = [None]
            for R, (it, p) in enumerate(rounds):
                for h in range(GRP):
                    _, t2, npt_t, stt_bi, t2_bi = front(it, GRP * p + h,
                                                        psg, None)
                    if prev_piece[0] is not None:
                        add_dep_helper(prev_piece[0].ins, stt_bi.ins,
                                       sync=False,
                                       reason="pin DVE blend/front interleave")
                        prev_piece[0] = None
                    tq.append((t2, npt_t, GRP * p + h))
                    if len(tq) > 1:
                        tanh_op(*tq.pop(0))
                    if bq:
                        it2, p2, k2 = bq.pop(0)
                        pbi = blend_piece(it2, p2, k2)
                        if not (k2 == 2 and it2 == n_iters - 1):
                            add_dep_helper(t2_bi.ins, pbi.ins, sync=False,
                                           reason="pin DVE piece after t2")
                            prev_piece[0] = pbi
                zp_s[(it, p)], npt_s[(it, p)] = zp[p], npt[p]
                if R >= lag - 1:
                    it2, p2 = rounds[R - (lag - 1)]
                    bq.extend([(it2, p2, k) for k in range(3)])
            while tq:
                tanh_op(*tq.pop(0))
            for k in range(max(0, len(rounds) - lag + 1), len(rounds)):
                it2, p2 = rounds[k]
                bq.extend([(it2, p2, j) for j in range(3)])
            while bq:
                blend_piece(*bq.pop(0))

    nc.compile()
    return nc


_NC_CACHE = {}


def kernel(**inputs):
    x = np.asarray(inputs["x"])
    wd, ba = _host_weights(inputs)
    xs = _host_x(x)

    if "prog" not in _NC_CACHE:
        _NC_CACHE["prog"] = _build_program()
    nc = _NC_CACHE["prog"]

    in_maps = []
    for c in range(N_CORES):
        m = {"x_il": xs[c]}
        m.update({k: wd[k] for k in ("wtb", "wt1", "a0t", "wat",
                                     "br", "bz", "binn", "bhn")})
        in_maps.append(m)

    res = run_bass_kernel_spmd(nc, in_maps, core_ids=list(range(N_CORES)))
    _NC_CACHE["last_result"] = res
    outs = []
    for c in range(N_CORES):
        yc = np.asarray(res.results[c]["y"], np.float32)  # [28, NCOL]
        oc = yc.reshape(4, 7, NCOL).transpose(0, 2, 1).reshape(BC, 7)
        outs.append(oc)
    out = np.concatenate(outs, 0).reshape(B, 7, 1) + np.float32(ba)
    return out.astype(np.float32)


if __name__ == "__main__":
    rng = np.random.default_rng(0)
    demo = {"x": rng.standard_normal((B, 18), dtype=np.float32)}
    for k, shp in [("Wj", (H, 2)), ("bj", (H,)), ("Wm", (H, H)), ("bm", (H,)),
                   ("Wih_j", (3 * H, 2 * H)), ("Whh_j", (3 * H, H)),
                   ("bih_j", (3 * H,)), ("bhh_j", (3 * H,)),
                   ("Wih_m", (3 * H, H)), ("Whh_m", (3 * H, H)),
                   ("bih_m", (3 * H,)), ("bhh_m", (3 * H,)),
                   ("Wa", (1, H)), ("ba", (1,))]:
        demo[k] = (rng.standard_normal(shp) * 0.1).astype(np.float32)
    y = kernel(**demo)
    print(y.shape, y.dtype)
